# revision 19
# baseline (speedup 1.0000x reference)
"""Trainium2 Bass kernel for nn_Decoder (attention decoder single step).

Sharding across 8 NeuronCores:
  - Attention: data-parallel over batch (16 batch elems / core). Encoder
    states are streamed once per core in two bf16 layouts so both the
    energy dot-product (contract over 2H) and the context weighted sum
    (contract over S) run on TensorE.
  - AllGather of per-core context rows -> full [128, 2048] on every core.
  - LSTM: tensor-parallel over hidden units (128 of 1024 units / core),
    weights pre-transposed + sliced on host.
  - AllGather of h_new unit-slices -> full h^T on every core.
  - FC: tensor-parallel over vocab (4000 of 32000 rows / core).

kernel(**inputs) takes the full unsharded inputs and returns
(predictions [128, 32000], h_new [1, 128, 1024], c_new [1, 128, 1024]).
"""

import numpy as np
import ml_dtypes

import jax
from jax.experimental.shard_map import shard_map
from jax.sharding import Mesh, NamedSharding, PartitionSpec

import concourse.bass as bass
import concourse.mybir as mybir
import concourse.tile as tile
from concourse import bacc, bass2jax
from concourse.bass_interp import get_hw_module

BF = ml_dtypes.bfloat16
F32 = mybir.dt.float32
BF16 = mybir.dt.bfloat16
I32 = mybir.dt.int32
F8 = mybir.dt.float8e4
F8NP = ml_dtypes.float8_e4m3
AF = mybir.ActivationFunctionType

import os
ENC_MODE = os.environ.get("KERNEL_ENC_MODE", "fp8")  # "fp8" | "bf16"
SIM_SINGLE = os.environ.get("KERNEL_SIM_SINGLE", "0") == "1"  # replace collectives with DMAs (for TimelineSim)
ENC_DT = F8 if ENC_MODE == "fp8" else BF16
ENC_NP = F8NP if ENC_MODE == "fp8" else BF
WENC_DT = F8 if ENC_MODE == "fp8" else BF16
WENC_SCALE = 64.0 if ENC_MODE == "fp8" else 1.0
EXP_DT = F8 if ENC_MODE == "fp8" else BF16
EXP_SCALE = 0.25 if ENC_MODE == "fp8" else 1.0

NCORES = 8
B = 128          # batch
BS = B // NCORES  # batch slice per core (16)
S = 512          # encoder sequence length
H = 1024         # hidden
H2 = 2 * H       # encoder feature dim (2048)
E = 512          # embedding dim
V = 32000        # vocab
VS = V // NCORES  # vocab slice per core (4000)
U = H // NCORES   # hidden-unit slice per core (128)
HC = H2 // 128    # h-chunks of 128 (16)
ST = S // 128     # s-tiles of 128 (4)
KR = (H2 + E) // 128  # rnn_in k-chunks (20)
KH = H // 128     # hidden k-chunks (8)
NVC = (VS + 511) // 512  # vocab free-dim chunks (8)


def _build(reps=1):
    nc = bacc.Bacc("TRN2", target_bir_lowering=False, debug=False,
                   enable_asserts=False, num_devices=NCORES)

    # ---- per-core inputs ----
    encN_d = nc.dram_tensor("encN", [BS, S, H2], ENC_DT, kind="ExternalInput")
    encT_d = nc.dram_tensor("encT", [BS, H2, S], ENC_DT, kind="ExternalInput")
    hT_d = nc.dram_tensor("hT", [H, B], BF16, kind="ExternalInput")
    hTs_d = nc.dram_tensor("hTs", [H, BS], BF16, kind="ExternalInput")
    cTs_d = nc.dram_tensor("cTs", [U, B], F32, kind="ExternalInput")
    xi_d = nc.dram_tensor("xi", [B, 1], I32, kind="ExternalInput")
    emb_d = nc.dram_tensor("embt", [V, E], F32, kind="ExternalInput")
    WENC_COLS = 16 * HC if ENC_MODE == "fp8" else HC
    wenc_d = nc.dram_tensor("wenc", [128, WENC_COLS], WENC_DT, kind="ExternalInput")
    wh_d = nc.dram_tensor("wh", [128, KH], BF16, kind="ExternalInput")
    be_d = nc.dram_tensor("be", [1, 1], F32, kind="ExternalInput")
    wih_d = nc.dram_tensor("wih", [H2 + E, 4 * U], BF16, kind="ExternalInput")
    whh_d = nc.dram_tensor("whh", [H, 4 * U], BF16, kind="ExternalInput")
    bias_d = nc.dram_tensor("bias", [U, 4], F32, kind="ExternalInput")
    wfc_d = nc.dram_tensor("wfc", [H, VS], BF16, kind="ExternalInput")
    bfc_d = nc.dram_tensor("bfc", [1, VS], BF16, kind="ExternalInput")
    idf_d = nc.dram_tensor("idf", [128, 128], F32, kind="ExternalInput")
    idb_d = nc.dram_tensor("idb", [128, 128], BF16, kind="ExternalInput")
    one_d = nc.dram_tensor("one", [1, 128], BF16, kind="ExternalInput")

    # ---- per-core outputs ----
    pred_d = nc.dram_tensor("pred", [B, VS], F32, kind="ExternalOutput")
    hout_d = nc.dram_tensor("hout", [U, B], F32, kind="ExternalOutput")
    cout_d = nc.dram_tensor("cout", [U, B], F32, kind="ExternalOutput")

    with tile.TileContext(nc) as tc:
        with tc.tile_pool(name="const", bufs=1) as cpool, \
             tc.tile_pool(name="wpool", bufs=1) as wpool, \
             tc.tile_pool(name="encn", bufs=3) as encn_pool, \
             tc.tile_pool(name="enct", bufs=3) as enct_pool, \
             tc.tile_pool(name="rowp", bufs=2) as rowp, \
             tc.tile_pool(name="work", bufs=1) as work, \
             tc.tile_pool(name="ps", bufs=4, space="PSUM") as ps, \
             tc.tile_pool(name="psc", bufs=1, space="PSUM") as psc, \
             tc.tile_pool(name="dram", bufs=1, space="DRAM") as dpool:
            for _rep in range(reps):
                _emit_body(nc, cpool, wpool, encn_pool, enct_pool, rowp, work,
                           ps, psc, dpool,
                           encN_d, encT_d, hT_d, hTs_d, cTs_d, xi_d, emb_d,
                           wenc_d, wh_d, be_d, wih_d, whh_d, bias_d, wfc_d,
                           bfc_d, idf_d, idb_d, one_d, pred_d, hout_d, cout_d)

    nc.compile()
    nc.m = get_hw_module(nc.m)
    return nc


def _emit_body(nc, cpool, wpool, encn_pool, enct_pool, rowp, work, ps, psc,
               dpool,
               encN_d, encT_d, hT_d, hTs_d, cTs_d, xi_d, emb_d, wenc_d, wh_d,
               be_d, wih_d, whh_d, bias_d, wfc_d, bfc_d, idf_d, idb_d, one_d,
               pred_d, hout_d, cout_d):
    import math
    EXP_BIAS = math.log(EXP_SCALE)

    # ---------- embedding gather kicked off first (Pool queue) ----------
    idx_sb = cpool.tile([B, 1], I32, name="idx_sb")
    nc.sync.dma_start(out=idx_sb[:], in_=xi_d[:])
    emb_sb = work.tile([B, E], F32, name="emb_sb")
    nc.gpsimd.indirect_dma_start(
        out=emb_sb[:], out_offset=None, in_=emb_d[:],
        in_offset=bass.IndirectOffsetOnAxis(ap=idx_sb[:, :1], axis=0))

    # ---------- encoder slice double-buffer stages ----------
    enc_tiles = {}

    def stage_load(b):
        encT_sb = enct_pool.tile([128, HC * S], ENC_DT, tag="enct",
                                 name="encT_sb")
        nc.sync.dma_start(
            out=encT_sb[:].rearrange("p (c s) -> p c s", s=S),
            in_=encT_d[b].rearrange("(c p) s -> p c s", p=128))
        encN_sb = encn_pool.tile([128, ST * H2], ENC_DT, tag="encn",
                                 name="encN_sb")
        nc.sync.dma_start(
            out=encN_sb[:].rearrange("p (c h) -> p c h", h=H2),
            in_=encN_d[b].rearrange("(c p) h -> p c h", p=128))
        enc_tiles[b] = (encT_sb, encN_sb)

    # ---------- constants / small weights (tiny; must precede enc DMAs) ----
    idf = cpool.tile([128, 128], F32, name="idf")
    nc.sync.dma_start(out=idf[:], in_=idf_d[:])
    idb = cpool.tile([128, 128], BF16, name="idb")
    nc.sync.dma_start(out=idb[:], in_=idb_d[:])
    id1 = cpool.tile([1, 1], F32, name="id1")
    nc.vector.memset(id1[:], 1.0)
    expb_sb = cpool.tile([1, 1], F32, name="expb_sb")
    nc.vector.memset(expb_sb[:], EXP_BIAS)
    ones_row = cpool.tile([1, 128], BF16, name="ones_row")
    nc.sync.dma_start(out=ones_row[:], in_=one_d[:])
    wenc_sb = cpool.tile([128, 16 * HC if ENC_MODE == "fp8" else HC], WENC_DT, name="wenc_sb")
    nc.sync.dma_start(out=wenc_sb[:], in_=wenc_d[:])
    wh_sb = cpool.tile([128, KH], BF16, name="wh_sb")
    nc.sync.dma_start(out=wh_sb[:], in_=wh_d[:])
    be_sb = cpool.tile([1, 1], F32, name="be_sb")
    nc.sync.dma_start(out=be_sb[:], in_=be_d[:])
    bias_sb = cpool.tile([U, 4], F32, name="bias_sb")
    nc.sync.dma_start(out=bias_sb[:], in_=bias_d[:])
    cT_sb = cpool.tile([U, B], F32, name="cT_sb")
    nc.sync.dma_start(out=cT_sb[:], in_=cTs_d[:])
    hTs_sb = cpool.tile([128, KH * BS], BF16, name="hTs_sb")
    for k in range(KH):
        nc.sync.dma_start(out=hTs_sb[:, k * BS:(k + 1) * BS],
                          in_=hTs_d[k * 128:(k + 1) * 128, :])

    # first two encoder slices next in the DMA queue
    stage_load(0)
    stage_load(1)

    # ---------- attention pipeline over the per-core batch slice ----------
    sums_sb = cpool.tile([1, BS], F32, name="sums_sb")
    inv_sb = cpool.tile([1, BS], F32, name="inv_sb")
    ctx_in = dpool.tile([BS, H2], BF16, name="ctx_in")   # collective bounce
    ctx_all = dpool.tile([B, H2], BF16, name="ctx_all")  # collective output
    e_tiles = {}

    def stage_energy(b):
        # energy row [1, S]: contract over 2H on TensorE
        e_ps = ps.tile([1, S], F32, tag="ps", name="eps")
        encT_sb = enc_tiles[b][0]
        if ENC_MODE == "fp8":
            # DoubleRow: 2 fp8 weights per PE cell, K-chunk pairs side by side
            wv = wenc_sb[:].rearrange("p (k i x) -> p k i x", k=HC // 2, i=2)
            ev = encT_sb[:].rearrange("p (c s) -> p c s", s=S)
            for k in range(HC // 2):
                nc.tensor.matmul(e_ps[:], wv[:, k, :, 0:1],
                                 ev[:, 2 * k:2 * k + 2, :],
                                 start=(k == 0), stop=(k == HC // 2 - 1),
                                 perf_mode=mybir.MatmulPerfMode.DoubleRow)
        else:
            for hc in range(HC):
                nc.tensor.matmul(e_ps[:], wenc_sb[:, hc:hc + 1],
                                 encT_sb[:, hc * S:(hc + 1) * S],
                                 start=(hc == 0), stop=(hc == HC - 1))
        e_tiles[b] = e_ps

    def stage_soft(b):
        # relu(e/scale + e_h[b]); exp(. + ln(EXP_SCALE)) with accumulated sum;
        # the EXP_SCALE factors cancel exactly at normalization time.
        e_ps = e_tiles.pop(b)
        relu_row = rowp.tile([1, S], F32, tag="relu", name="relu_row")
        nc.scalar.activation(relu_row[:], e_ps[:], AF.Relu,
                             bias=ehrow_sb[0:1, b:b + 1],
                             scale=1.0 / WENC_SCALE)
        exp_row = rowp.tile([1, S], F32, tag="exp", name="exp_row")
        nc.scalar.activation(exp_row[:], relu_row[:], AF.Exp,
                             bias=expb_sb[0:1, 0:1],
                             accum_out=sums_sb[0:1, b:b + 1])
        nc.vector.reciprocal(inv_sb[0:1, b:b + 1], sums_sb[0:1, b:b + 1])
        # transpose exp row into columns [128(s), ST]
        x_ps = ps.tile([128, ST], F32, tag="ps", name="xps")
        for st in range(ST):
            nc.tensor.transpose(x_ps[:, st:st + 1],
                                exp_row[0:1, st * 128:(st + 1) * 128], id1[:])
        if ENC_MODE == "fp8":
            expc_sb = rowp.tile([128, 16 * ST], EXP_DT, tag="expc",
                                name="expc_sb")
            nc.vector.tensor_copy(
                out=expc_sb[:].rearrange("p (j i x) -> p j i x",
                                         j=ST // 2, i=2)[:, :, :, 0:1],
                in_=x_ps[:].rearrange("p (j i) -> p j i", j=ST // 2)[:, :, :, None])
        else:
            expc_sb = rowp.tile([128, ST], EXP_DT, tag="expc", name="expc_sb")
            nc.vector.tensor_copy(out=expc_sb[:], in_=x_ps[:])
        return expc_sb

    def stage_ctx(b, expc_sb):
        # context row [1, 2048]: contract over S on TensorE; normalize by
        # 1/sum during the single PSUM->SBUF copy on ScalarE
        encN_sb = enc_tiles[b][1]
        c_ps = psc.tile([1, H2], F32, tag="cps", name="cps")
        if ENC_MODE == "fp8":
            xv = expc_sb[:].rearrange("p (j i x) -> p j i x", j=ST // 2, i=2)
            nv = encN_sb[:].rearrange("p (st h) -> p st h", st=ST)
            for nk in range(H2 // 512):
                for j in range(ST // 2):
                    nc.tensor.matmul(
                        c_ps[0:1, nk * 512:(nk + 1) * 512], xv[:, j, :, 0:1],
                        nv[:, 2 * j:2 * j + 2, nk * 512:(nk + 1) * 512],
                        start=(j == 0), stop=(j == ST // 2 - 1),
                        perf_mode=mybir.MatmulPerfMode.DoubleRow)
        else:
            for nk in range(H2 // 512):
                for st in range(ST):
                    nc.tensor.matmul(
                        c_ps[0:1, nk * 512:(nk + 1) * 512],
                        expc_sb[:, st:st + 1],
                        encN_sb[:, st * H2 + nk * 512: st * H2 + (nk + 1) * 512],
                        start=(st == 0), stop=(st == ST - 1))
        ctx_row = rowp.tile([1, H2], BF16, tag="ctxr", name="ctx_row")
        nc.scalar.mul(ctx_row[:], c_ps[:], inv_sb[0:1, b:b + 1])
        nc.sync.dma_start(out=ctx_in[b:b + 1, :], in_=ctx_row[:])
        del enc_tiles[b]

    # weight loads to interleave with the encoder stream (DMA has slack in
    # the PE-bound attention phase): need-ordered wih -> whh -> hT -> wfc
    wih_sb = wpool.tile([128, KR * 4 * U], BF16, name="wih_sb")
    whh_sb = wpool.tile([128, KH * 4 * U], BF16, name="whh_sb")
    hT_sb = cpool.tile([128, KH * B], BF16, name="hT_sb")
    wfc_sb = wpool.tile([128, KH * VS], BF16, name="wfc_sb")
    bfc_sb = wpool.tile([1, VS], BF16, name="bfc_sb")
    interleaved = []
    for k in range(KR):
        interleaved.append((wih_sb[:, k * 4 * U:(k + 1) * 4 * U],
                            wih_d[k * 128:(k + 1) * 128, :]))
    for k in range(KH):
        interleaved.append((whh_sb[:, k * 4 * U:(k + 1) * 4 * U],
                            whh_d[k * 128:(k + 1) * 128, :]))
    for k in range(KH):
        interleaved.append((hT_sb[:, k * B:(k + 1) * B],
                            hT_d[k * 128:(k + 1) * 128, :]))
    per_iter = (len(interleaved) + BS - 1) // BS

    stage_energy(0)

    # ---------- e_h row + emb transposes (PE order: after energy(0)) ----------
    eh_ps = ps.tile([1, BS], F32, tag="ps", name="ehps")
    for k in range(KH):
        nc.tensor.matmul(eh_ps[:], wh_sb[:, k:k + 1],
                         hTs_sb[:, k * BS:(k + 1) * BS],
                         start=(k == 0), stop=(k == KH - 1))
    ehrow_sb = cpool.tile([1, BS], F32, name="ehrow_sb")
    nc.vector.tensor_scalar_add(ehrow_sb[:], eh_ps[:], be_sb[0:1, 0:1])
    # rnn_in^T tile: k-chunks 0..15 = context^T (filled later), 16..19 = emb^T
    rnnT_sb = work.tile([128, KR * B], BF16, name="rnnT_sb")
    for ec in range(E // 128):
        pt = ps.tile([128, 128], F32, tag="ps", name="ptf")
        nc.tensor.transpose(pt[:], emb_sb[:, ec * 128:(ec + 1) * 128], idf[:])
        nc.vector.tensor_copy(
            out=rnnT_sb[:, (HC + ec) * B:(HC + ec + 1) * B], in_=pt[:])

    for b in range(BS):
        if b + 1 < BS:
            stage_energy(b + 1)
        expc = stage_soft(b)
        stage_ctx(b, expc)
        if b + 2 < BS:
            stage_load(b + 2)
        for out_ap, in_ap in interleaved[b * per_iter:(b + 1) * per_iter]:
            nc.sync.dma_start(out=out_ap, in_=in_ap)

    # ---------- LSTM gates: emb + h_prev accumulation (collective-overlap) ----
    # All 4 gates share one PSUM bank [U, 4*B]; only the very first matmul
    # uses start=True, per-element has_written handles the rest.
    gps_all = ps.tile([U, 4 * B], F32, tag="ps", name="gps_all")
    first_mm = True
    for g in range(4):
        for k in range(HC, KR):
            nc.tensor.matmul(
                gps_all[:, g * B:(g + 1) * B],
                wih_sb[:, k * 4 * U + g * U: k * 4 * U + (g + 1) * U],
                rnnT_sb[:, k * B:(k + 1) * B], start=first_mm, stop=False,
                skip_group_check=True)
            first_mm = False
        for k in range(KH):
            nc.tensor.matmul(
                gps_all[:, g * B:(g + 1) * B],
                whh_sb[:, k * 4 * U + g * U: k * 4 * U + (g + 1) * U],
                hT_sb[:, k * B:(k + 1) * B], start=False, stop=False,
                skip_group_check=True)

    # ---------- gather context to all cores; build rnn_in^T ----------
    if SIM_SINGLE:
        for r in range(NCORES):
            nc.sync.dma_start(out=ctx_all[r * BS:(r + 1) * BS, :], in_=ctx_in[:])
    else:
        nc.gpsimd.collective_compute(
            "AllGather", mybir.AluOpType.bypass,
            replica_groups=[list(range(NCORES))],
            ins=[ctx_in.opt()], outs=[ctx_all.opt()])
    ctx_sb = work.tile([B, H2], BF16, name="ctx_sb")
    nc.sync.dma_start(out=ctx_sb[:], in_=ctx_all[:])
    for kc in range(HC):
        ptb = psc.tile([128, 128], BF16, tag="cps", name="ptb")
        nc.tensor.transpose(ptb[:], ctx_sb[:, kc * 128:(kc + 1) * 128], idb[:])
        nc.vector.tensor_copy(out=rnnT_sb[:, kc * B:(kc + 1) * B], in_=ptb[:])

    # ---------- FC weights load (after ctx-gather DMAs in queue order) ------
    nc.sync.dma_start(out=bfc_sb[:], in_=bfc_d[:])
    for k in range(KH):
        nc.sync.dma_start(out=wfc_sb[:, k * VS:(k + 1) * VS],
                          in_=wfc_d[k * 128:(k + 1) * 128, :])

    # ---------- LSTM gates: context accumulation + activations ----------
    gate_sb = []
    gate_fn = [AF.Sigmoid, AF.Sigmoid, AF.Tanh, AF.Sigmoid]
    for g in range(4):
        for k in range(HC):
            nc.tensor.matmul(
                gps_all[:, g * B:(g + 1) * B],
                wih_sb[:, k * 4 * U + g * U: k * 4 * U + (g + 1) * U],
                rnnT_sb[:, k * B:(k + 1) * B], start=False,
                stop=(g == 3 and k == HC - 1), skip_group_check=True)
    for g in range(4):
        gs = work.tile([U, B], F32, name=f"gate{g}", tag=f"gate{g}")
        nc.scalar.activation(gs[:], gps_all[:, g * B:(g + 1) * B], gate_fn[g],
                             bias=bias_sb[:, g:g + 1], scale=1.0)
        gate_sb.append(gs)

    fc_sb = work.tile([U, B], F32, name="fc_sb")
    nc.vector.tensor_mul(fc_sb[:], gate_sb[1][:], cT_sb[:])
    ig_sb = work.tile([U, B], F32, name="ig_sb")
    nc.vector.tensor_mul(ig_sb[:], gate_sb[0][:], gate_sb[2][:])
    cnew_sb = work.tile([U, B], F32, name="cnew_sb")
    nc.vector.tensor_add(cnew_sb[:], fc_sb[:], ig_sb[:])
    nc.sync.dma_start(out=cout_d[:], in_=cnew_sb[:])
    tanhc_sb = work.tile([U, B], F32, name="tanhc_sb")
    nc.scalar.activation(tanhc_sb[:], cnew_sb[:], AF.Tanh)
    hnew_sb = work.tile([U, B], F32, name="hnew_sb")
    nc.vector.tensor_mul(hnew_sb[:], gate_sb[3][:], tanhc_sb[:])
    nc.sync.dma_start(out=hout_d[:], in_=hnew_sb[:])

    # ---------- gather h_new to all cores ----------
    hnewb_sb = work.tile([U, B], BF16, name="hnewb_sb")
    nc.vector.tensor_copy(out=hnewb_sb[:], in_=hnew_sb[:])
    h_in = dpool.tile([U, B], BF16, name="h_in")
    h_all = dpool.tile([H, B], BF16, name="h_all")
    nc.sync.dma_start(out=h_in[:], in_=hnewb_sb[:])
    if SIM_SINGLE:
        for r in range(NCORES):
            nc.sync.dma_start(out=h_all[r * U:(r + 1) * U, :], in_=h_in[:])
    else:
        nc.gpsimd.collective_compute(
            "AllGather", mybir.AluOpType.bypass,
            replica_groups=[list(range(NCORES))],
            ins=[h_in.opt()], outs=[h_all.opt()])
    hTn_sb = work.tile([128, KH * B], BF16, name="hTn_sb")
    for k in range(KH):
        nc.sync.dma_start(out=hTn_sb[:, k * B:(k + 1) * B],
                          in_=h_all[k * 128:(k + 1) * 128, :])

    # ---------- FC (vocab-parallel): logits [b, v_slice] ----------
    for vc in range(NVC):
        v0 = vc * 512
        vn = min(512, VS - v0)
        f_ps = ps.tile([B, 512], F32, tag="ps", name="fps")
        nc.tensor.matmul(f_ps[:, :vn], ones_row[0:1, :],
                         bfc_sb[0:1, v0:v0 + vn], start=True, stop=False)
        for k in range(KH):
            nc.tensor.matmul(
                f_ps[:, :vn], hTn_sb[:, k * B:(k + 1) * B],
                wfc_sb[:, k * VS + v0: k * VS + v0 + vn],
                start=False, stop=(k == KH - 1))
        o_sb = rowp.tile([B, 512], F32, tag="osb", name="o_sb")
        nc.scalar.copy(o_sb[:, :vn], f_ps[:, :vn])
        nc.sync.dma_start(out=pred_d[:, v0:v0 + vn], in_=o_sb[:, :vn])


class _Runner:
    """Caches the jitted shard_map executable for a compiled Bass program."""

    def __init__(self, nc):
        bass2jax.install_neuronx_cc_hook()
        self.nc = nc
        assert nc.dbg_addr is None
        part_name = (nc.partition_id_tensor.name
                     if nc.partition_id_tensor else None)
        in_names, out_names, out_avals, zero_shapes = [], [], [], []
        for alloc in nc.m.functions[0].allocations:
            if not isinstance(alloc, mybir.MemoryLocationSet):
                continue
            name = alloc.memorylocations[0].name
            if alloc.kind == "ExternalInput":
                if name != part_name:
                    in_names.append(name)
            elif alloc.kind == "ExternalOutput":
                out_names.append(name)
                shape = tuple(alloc.tensor_shape)
                dtype = mybir.dt.np(alloc.dtype)
                out_avals.append(jax.core.ShapedArray(shape, dtype))
                zero_shapes.append((shape, dtype))
        self.in_names = in_names
        self.out_names = out_names
        self.zero_shapes = zero_shapes
        n_params = len(in_names)
        n_outs = len(out_names)
        all_names = list(in_names) + list(out_names)
        if part_name is not None:
            all_names.append(part_name)
        all_names = tuple(all_names)

        def _body(*args):
            operands = list(args)
            if part_name is not None:
                operands.append(bass2jax.partition_id_tensor())
            outs = bass2jax._bass_exec_p.bind(
                *operands, out_avals=tuple(out_avals), in_names=all_names,
                out_names=tuple(out_names), lowering_input_output_aliases=(),
                sim_require_finite=True, sim_require_nnan=True, nc=nc)
            return tuple(outs)

        devices = jax.devices()[:NCORES]
        assert len(devices) == NCORES
        self.mesh = Mesh(np.asarray(devices), ("core",))
        in_specs = (PartitionSpec("core"),) * (n_params + n_outs)
        out_specs = (PartitionSpec("core"),) * n_outs
        self._fn = jax.jit(
            shard_map(_body, mesh=self.mesh, in_specs=in_specs,
                      out_specs=out_specs, check_rep=False),
            donate_argnums=tuple(range(n_params, n_params + n_outs)),
            keep_unused=True)

    def concat_inputs(self, in_maps):
        return [np.concatenate([np.asarray(in_maps[c][n]) for c in range(NCORES)],
                               axis=0) for n in self.in_names]

    def device_put_inputs(self, in_maps):
        sh = NamedSharding(self.mesh, PartitionSpec("core"))
        return [jax.device_put(a, sh) for a in self.concat_inputs(in_maps)]

    def zeros(self):
        return [np.zeros((NCORES * s[0], *s[1:]), d) for s, d in self.zero_shapes]

    def run(self, concat_in):
        outs = self._fn(*concat_in, *self.zeros())
        return [o.block_until_ready() for o in outs]

    def results(self, outs):
        out_np = [np.asarray(o) for o in outs]
        return [
            {n: out_np[i].reshape(NCORES, *self.zero_shapes[i][0])[c]
             for i, n in enumerate(self.out_names)}
            for c in range(NCORES)
        ]


_RUNNERS = {}


def get_runner(reps=1):
    if reps not in _RUNNERS:
        _RUNNERS[reps] = _Runner(_build(reps))
    return _RUNNERS[reps]


def _prep_inputs(x, encoder_states, hidden, cell, emb_table, W_e, b_e,
                 W_ih, W_hh, b_ih, b_hh, W_fc, b_fc):
    """Host-side sharding + layout prep. Returns in_maps for the 8 cores."""
    x = np.asarray(x).astype(np.int32).reshape(B, 1)
    enc = np.asarray(encoder_states, dtype=np.float32)
    hidden = np.asarray(hidden, dtype=np.float32)
    cell = np.asarray(cell, dtype=np.float32)
    emb_table = np.ascontiguousarray(np.asarray(emb_table, dtype=np.float32))
    W_e = np.asarray(W_e, dtype=np.float32)
    b_e = np.asarray(b_e, dtype=np.float32)
    W_ih = np.asarray(W_ih, dtype=np.float32)
    W_hh = np.asarray(W_hh, dtype=np.float32)
    b_ih = np.asarray(b_ih, dtype=np.float32)
    b_hh = np.asarray(b_hh, dtype=np.float32)
    W_fc = np.asarray(W_fc, dtype=np.float32)
    b_fc = np.asarray(b_fc, dtype=np.float32)

    enc_q = enc.astype(ENC_NP)
    encN_all = np.ascontiguousarray(enc_q.transpose(1, 0, 2))  # [B, S, H2]
    encT_all = np.ascontiguousarray(enc_q.transpose(1, 2, 0))  # [B, H2, S]

    hT = np.ascontiguousarray(hidden[0].T)          # [H, B] f32
    hT_bf = hT.astype(BF)
    cT = np.ascontiguousarray(cell[0].T)            # [H, B] f32
    wh_t = np.ascontiguousarray(W_e[:H, 0].reshape(KH, 128).T).astype(BF)
    wenc_chunks = (W_e[H:, 0] * WENC_SCALE).reshape(HC, 128)
    if ENC_MODE == 'fp8':
        wenc_t = np.zeros((128, 16 * HC), dtype=F8NP)
        for k in range(HC // 2):
            wenc_t[:, 32 * k] = wenc_chunks[2 * k].astype(F8NP)
            wenc_t[:, 32 * k + 16] = wenc_chunks[2 * k + 1].astype(F8NP)
    else:
        wenc_t = np.ascontiguousarray(wenc_chunks.T).astype(BF)
    be = b_e.reshape(1, 1)
    bias_all = b_ih + b_hh
    idf = np.eye(128, dtype=np.float32)
    idb = np.eye(128, dtype=BF)
    one_row = np.ones((1, 128), dtype=BF)

    in_maps = []
    for c in range(NCORES):
        b0 = c * BS
        u0 = c * U
        v0 = c * VS
        rows = np.concatenate([np.arange(u0, u0 + U) + g * H for g in range(4)])
        in_maps.append({
            "encN": encN_all[b0:b0 + BS],
            "encT": encT_all[b0:b0 + BS],
            "hT": hT_bf,
            "hTs": np.ascontiguousarray(hT_bf[:, b0:b0 + BS]),
            "cTs": np.ascontiguousarray(cT[u0:u0 + U, :]),
            "xi": x,
            "embt": emb_table,
            "wenc": wenc_t,
            "wh": wh_t,
            "be": be,
            "wih": np.ascontiguousarray(W_ih[rows].T).astype(BF),
            "whh": np.ascontiguousarray(W_hh[rows].T).astype(BF),
            "bias": np.ascontiguousarray(bias_all[rows].reshape(4, U).T),
            "wfc": np.ascontiguousarray(W_fc[v0:v0 + VS].T).astype(BF),
            "bfc": b_fc[v0:v0 + VS].reshape(1, VS).astype(BF),
            "idf": idf,
            "idb": idb,
            "one": one_row,
        })
    return in_maps


def kernel(**inputs):
    runner = get_runner(reps=1)
    in_maps = _prep_inputs(**inputs)
    outs = runner.run(runner.concat_inputs(in_maps))
    return assemble(runner.results(outs))


def assemble(results):
    preds = np.concatenate([results[c]["pred"] for c in range(NCORES)], axis=1)
    hT_new = np.concatenate([results[c]["hout"] for c in range(NCORES)], axis=0)
    cT_new = np.concatenate([results[c]["cout"] for c in range(NCORES)], axis=0)
    h_new = np.ascontiguousarray(hT_new.T)[None]
    c_new = np.ascontiguousarray(cT_new.T)[None]
    return preds, h_new, c_new


# revision 20
# speedup vs baseline: 1.2904x; 1.2904x over previous
"""Trainium2 Bass kernel for nn_Decoder (attention decoder single step).

Sharding across 8 NeuronCores:
  - Attention: data-parallel over batch (16 batch elems / core). Encoder
    states are streamed once per core in two bf16 layouts so both the
    energy dot-product (contract over 2H) and the context weighted sum
    (contract over S) run on TensorE.
  - AllGather of per-core context rows -> full [128, 2048] on every core.
  - LSTM: tensor-parallel over hidden units (128 of 1024 units / core),
    weights pre-transposed + sliced on host.
  - AllGather of h_new unit-slices -> full h^T on every core.
  - FC: tensor-parallel over vocab (4000 of 32000 rows / core).

kernel(**inputs) takes the full unsharded inputs and returns
(predictions [128, 32000], h_new [1, 128, 1024], c_new [1, 128, 1024]).
"""

import numpy as np
import ml_dtypes

import jax
from jax.experimental.shard_map import shard_map
from jax.sharding import Mesh, NamedSharding, PartitionSpec

import concourse.bass as bass
import concourse.mybir as mybir
import concourse.tile as tile
from concourse import bacc, bass2jax
from concourse.bass_interp import get_hw_module

BF = ml_dtypes.bfloat16
F32 = mybir.dt.float32
BF16 = mybir.dt.bfloat16
I32 = mybir.dt.int32
F8 = mybir.dt.float8e4
F8NP = ml_dtypes.float8_e4m3
AF = mybir.ActivationFunctionType

import os
ENC_MODE = os.environ.get("KERNEL_ENC_MODE", "fp8")  # "fp8" | "bf16"
SIM_SINGLE = os.environ.get("KERNEL_SIM_SINGLE", "0") == "1"  # replace collectives with DMAs (for TimelineSim)
ENC_DT = F8 if ENC_MODE == "fp8" else BF16
ENC_NP = F8NP if ENC_MODE == "fp8" else BF
WENC_DT = F8 if ENC_MODE == "fp8" else BF16
WENC_SCALE = 64.0 if ENC_MODE == "fp8" else 1.0
EXP_DT = F8 if ENC_MODE == "fp8" else BF16
EXP_SCALE = 0.25 if ENC_MODE == "fp8" else 1.0

NCORES = 8
B = 128          # batch
BS = B // NCORES  # batch slice per core (16)
S = 512          # encoder sequence length
H = 1024         # hidden
H2 = 2 * H       # encoder feature dim (2048)
E = 512          # embedding dim
V = 32000        # vocab
VS = V // NCORES  # vocab slice per core (4000)
U = H // NCORES   # hidden-unit slice per core (128)
HC = H2 // 128    # h-chunks of 128 (16)
ST = S // 128     # s-tiles of 128 (4)
KR = (H2 + E) // 128  # rnn_in k-chunks (20)
KH = H // 128     # hidden k-chunks (8)
NVC = (VS + 511) // 512  # vocab free-dim chunks (8)


def _build(reps=1):
    nc = bacc.Bacc("TRN2", target_bir_lowering=False, debug=False,
                   enable_asserts=False, num_devices=NCORES)

    # ---- per-core inputs ----
    encN_d = nc.dram_tensor("encN", [BS, S, H2], ENC_DT, kind="ExternalInput")
    encT_d = nc.dram_tensor("encT", [BS, H2, S], ENC_DT, kind="ExternalInput")
    hT_d = nc.dram_tensor("hT", [H, B], BF16, kind="ExternalInput")
    hTs_d = nc.dram_tensor("hTs", [H, BS], BF16, kind="ExternalInput")
    cTs_d = nc.dram_tensor("cTs", [U, B], F32, kind="ExternalInput")
    xi_d = nc.dram_tensor("xi", [B, 1], I32, kind="ExternalInput")
    emb_d = nc.dram_tensor("embt", [V, E], F32, kind="ExternalInput")
    WENC_COLS = 16 * HC if ENC_MODE == "fp8" else HC
    wenc_d = nc.dram_tensor("wenc", [128, WENC_COLS], WENC_DT, kind="ExternalInput")
    wh_d = nc.dram_tensor("wh", [128, KH], BF16, kind="ExternalInput")
    be_d = nc.dram_tensor("be", [1, 1], F32, kind="ExternalInput")
    wih_d = nc.dram_tensor("wih", [H2 + E, 4 * U], BF16, kind="ExternalInput")
    whh_d = nc.dram_tensor("whh", [H, 4 * U], BF16, kind="ExternalInput")
    bias_d = nc.dram_tensor("bias", [U, 4], F32, kind="ExternalInput")
    wfc_d = nc.dram_tensor("wfc", [H, VS], BF16, kind="ExternalInput")
    bfc_d = nc.dram_tensor("bfc", [1, VS], BF16, kind="ExternalInput")
    idf_d = nc.dram_tensor("idf", [128, 128], F32, kind="ExternalInput")
    idb_d = nc.dram_tensor("idb", [128, 128], BF16, kind="ExternalInput")
    one_d = nc.dram_tensor("one", [1, 128], BF16, kind="ExternalInput")

    # ---- per-core outputs ----
    pred_d = nc.dram_tensor("pred", [B, VS], F32, kind="ExternalOutput")
    hout_d = nc.dram_tensor("hout", [U, B], F32, kind="ExternalOutput")
    cout_d = nc.dram_tensor("cout", [U, B], F32, kind="ExternalOutput")

    with tile.TileContext(nc) as tc:
        with tc.tile_pool(name="const", bufs=1) as cpool, \
             tc.tile_pool(name="wpool", bufs=1) as wpool, \
             tc.tile_pool(name="encn", bufs=4) as encn_pool, \
             tc.tile_pool(name="enct", bufs=4) as enct_pool, \
             tc.tile_pool(name="rowp", bufs=2) as rowp, \
             tc.tile_pool(name="work", bufs=1) as work, \
             tc.tile_pool(name="ps", bufs=4, space="PSUM") as ps, \
             tc.tile_pool(name="psc", bufs=1, space="PSUM") as psc, \
             tc.tile_pool(name="dram", bufs=1, space="DRAM") as dpool:
            for _rep in range(reps):
                _emit_body(nc, cpool, wpool, encn_pool, enct_pool, rowp, work,
                           ps, psc, dpool,
                           encN_d, encT_d, hT_d, hTs_d, cTs_d, xi_d, emb_d,
                           wenc_d, wh_d, be_d, wih_d, whh_d, bias_d, wfc_d,
                           bfc_d, idf_d, idb_d, one_d, pred_d, hout_d, cout_d)

    nc.compile()
    nc.m = get_hw_module(nc.m)
    return nc


def _emit_body(nc, cpool, wpool, encn_pool, enct_pool, rowp, work, ps, psc,
               dpool,
               encN_d, encT_d, hT_d, hTs_d, cTs_d, xi_d, emb_d, wenc_d, wh_d,
               be_d, wih_d, whh_d, bias_d, wfc_d, bfc_d, idf_d, idb_d, one_d,
               pred_d, hout_d, cout_d):
    import math
    EXP_BIAS = math.log(EXP_SCALE)

    # ---------- embedding gather kicked off first (Pool queue) ----------
    idx_sb = cpool.tile([B, 1], I32, name="idx_sb")
    nc.sync.dma_start(out=idx_sb[:], in_=xi_d[:])
    emb_sb = work.tile([B, E], F32, name="emb_sb")
    nc.gpsimd.indirect_dma_start(
        out=emb_sb[:], out_offset=None, in_=emb_d[:],
        in_offset=bass.IndirectOffsetOnAxis(ap=idx_sb[:, :1], axis=0))

    # ---------- encoder slice double-buffer stages ----------
    enc_tiles = {}

    def stage_load(b):
        encT_sb = enct_pool.tile([128, HC * S], ENC_DT, tag="enct",
                                 name="encT_sb")
        nc.sync.dma_start(
            out=encT_sb[:].rearrange("p (c s) -> p c s", s=S),
            in_=encT_d[b].rearrange("(c p) s -> p c s", p=128))
        encN_sb = encn_pool.tile([128, ST * H2], ENC_DT, tag="encn",
                                 name="encN_sb")
        nc.sync.dma_start(
            out=encN_sb[:].rearrange("p (c h) -> p c h", h=H2),
            in_=encN_d[b].rearrange("(c p) h -> p c h", p=128))
        enc_tiles[b] = (encT_sb, encN_sb)

    # ---------- constants / small weights (tiny; must precede enc DMAs) ----
    idf = cpool.tile([128, 128], F32, name="idf")
    nc.sync.dma_start(out=idf[:], in_=idf_d[:])
    idb = cpool.tile([128, 128], BF16, name="idb")
    nc.sync.dma_start(out=idb[:], in_=idb_d[:])
    id1 = cpool.tile([1, 1], F32, name="id1")
    nc.vector.memset(id1[:], 1.0)
    expb_sb = cpool.tile([1, 1], F32, name="expb_sb")
    nc.vector.memset(expb_sb[:], EXP_BIAS)
    ones_row = cpool.tile([1, 128], BF16, name="ones_row")
    nc.sync.dma_start(out=ones_row[:], in_=one_d[:])
    wenc_sb = cpool.tile([128, 16 * HC if ENC_MODE == "fp8" else HC], WENC_DT, name="wenc_sb")
    nc.sync.dma_start(out=wenc_sb[:], in_=wenc_d[:])
    wh_sb = cpool.tile([128, KH], BF16, name="wh_sb")
    nc.sync.dma_start(out=wh_sb[:], in_=wh_d[:])
    be_sb = cpool.tile([1, 1], F32, name="be_sb")
    nc.sync.dma_start(out=be_sb[:], in_=be_d[:])
    bias_sb = cpool.tile([U, 4], F32, name="bias_sb")
    nc.sync.dma_start(out=bias_sb[:], in_=bias_d[:])
    cT_sb = cpool.tile([U, B], F32, name="cT_sb")
    nc.sync.dma_start(out=cT_sb[:], in_=cTs_d[:])
    hTs_sb = cpool.tile([128, KH * BS], BF16, name="hTs_sb")
    for k in range(KH):
        nc.sync.dma_start(out=hTs_sb[:, k * BS:(k + 1) * BS],
                          in_=hTs_d[k * 128:(k + 1) * 128, :])

    # first encoder slices head the DMA queue
    stage_load(0)
    stage_load(1)
    stage_load(2)

    # ---------- attention pipeline over the per-core batch slice ----------
    sums_sb = cpool.tile([1, BS], F32, name="sums_sb")
    inv_sb = cpool.tile([1, BS], F32, name="inv_sb")
    ctx_in = dpool.tile([BS, H2], BF16, name="ctx_in")   # collective bounce
    ctx_all = dpool.tile([B, H2], BF16, name="ctx_all")  # collective output
    e_tiles = {}

    def stage_energy(b):
        # energy row [1, S]: contract over 2H on TensorE
        e_ps = ps.tile([1, S], F32, tag="ps", name="eps")
        encT_sb = enc_tiles[b][0]
        if ENC_MODE == "fp8":
            # DoubleRow: 2 fp8 weights per PE cell, K-chunk pairs side by side
            wv = wenc_sb[:].rearrange("p (k i x) -> p k i x", k=HC // 2, i=2)
            ev = encT_sb[:].rearrange("p (c s) -> p c s", s=S)
            for k in range(HC // 2):
                nc.tensor.matmul(e_ps[:], wv[:, k, :, 0:1],
                                 ev[:, 2 * k:2 * k + 2, :],
                                 start=(k == 0), stop=(k == HC // 2 - 1),
                                 perf_mode=mybir.MatmulPerfMode.DoubleRow)
        else:
            for hc in range(HC):
                nc.tensor.matmul(e_ps[:], wenc_sb[:, hc:hc + 1],
                                 encT_sb[:, hc * S:(hc + 1) * S],
                                 start=(hc == 0), stop=(hc == HC - 1))
        e_tiles[b] = e_ps

    def stage_soft(b):
        # relu(e/scale + e_h[b]); exp(. + ln(EXP_SCALE)) with accumulated sum;
        # the EXP_SCALE factors cancel exactly at normalization time.
        e_ps = e_tiles.pop(b)
        relu_row = rowp.tile([1, S], F32, tag="relu", name="relu_row")
        nc.scalar.activation(relu_row[:], e_ps[:], AF.Relu,
                             bias=ehrow_sb[0:1, b:b + 1],
                             scale=1.0 / WENC_SCALE)
        exp_row = rowp.tile([1, S], F32, tag="exp", name="exp_row")
        nc.scalar.activation(exp_row[:], relu_row[:], AF.Exp,
                             bias=expb_sb[0:1, 0:1],
                             accum_out=sums_sb[0:1, b:b + 1])
        nc.vector.reciprocal(inv_sb[0:1, b:b + 1], sums_sb[0:1, b:b + 1])
        # transpose exp row into columns [128(s), ST]
        x_ps = ps.tile([128, ST], F32, tag="ps", name="xps")
        for st in range(ST):
            nc.tensor.transpose(x_ps[:, st:st + 1],
                                exp_row[0:1, st * 128:(st + 1) * 128], id1[:])
        if ENC_MODE == "fp8":
            expc_sb = rowp.tile([128, 16 * ST], EXP_DT, tag="expc",
                                name="expc_sb")
            nc.vector.tensor_copy(
                out=expc_sb[:].rearrange("p (j i x) -> p j i x",
                                         j=ST // 2, i=2)[:, :, :, 0:1],
                in_=x_ps[:].rearrange("p (j i) -> p j i", j=ST // 2)[:, :, :, None])
        else:
            expc_sb = rowp.tile([128, ST], EXP_DT, tag="expc", name="expc_sb")
            nc.vector.tensor_copy(out=expc_sb[:], in_=x_ps[:])
        return expc_sb

    def stage_ctx(b, expc_sb):
        # context row [1, 2048]: contract over S on TensorE; normalize by
        # 1/sum during the single PSUM->SBUF copy on ScalarE
        encN_sb = enc_tiles[b][1]
        c_ps = psc.tile([1, H2], F32, tag="cps", name="cps")
        if ENC_MODE == "fp8":
            xv = expc_sb[:].rearrange("p (j i x) -> p j i x", j=ST // 2, i=2)
            nv = encN_sb[:].rearrange("p (st h) -> p st h", st=ST)
            for nk in range(H2 // 512):
                for j in range(ST // 2):
                    nc.tensor.matmul(
                        c_ps[0:1, nk * 512:(nk + 1) * 512], xv[:, j, :, 0:1],
                        nv[:, 2 * j:2 * j + 2, nk * 512:(nk + 1) * 512],
                        start=(j == 0), stop=(j == ST // 2 - 1),
                        perf_mode=mybir.MatmulPerfMode.DoubleRow)
        else:
            for nk in range(H2 // 512):
                for st in range(ST):
                    nc.tensor.matmul(
                        c_ps[0:1, nk * 512:(nk + 1) * 512],
                        expc_sb[:, st:st + 1],
                        encN_sb[:, st * H2 + nk * 512: st * H2 + (nk + 1) * 512],
                        start=(st == 0), stop=(st == ST - 1))
        ctx_row = rowp.tile([1, H2], BF16, tag="ctxr", name="ctx_row")
        nc.scalar.mul(ctx_row[:], c_ps[:], inv_sb[0:1, b:b + 1])
        nc.sync.dma_start(out=ctx_in[b:b + 1, :], in_=ctx_row[:])
        del enc_tiles[b]

    # weight loads to interleave with the encoder stream (DMA has slack in
    # the PE-bound attention phase): need-ordered wih -> whh -> hT -> wfc
    wih_sb = wpool.tile([128, KR * 4 * U], BF16, name="wih_sb")
    whh_sb = wpool.tile([128, KH * 4 * U], BF16, name="whh_sb")
    hT_sb = cpool.tile([128, KH * B], BF16, name="hT_sb")
    wfc_sb = wpool.tile([128, KH * VS], BF16, name="wfc_sb")
    bfc_sb = wpool.tile([1, VS], BF16, name="bfc_sb")
    interleaved = []
    for k in range(KR):
        interleaved.append((wih_sb[:, k * 4 * U:(k + 1) * 4 * U],
                            wih_d[k * 128:(k + 1) * 128, :]))
    for k in range(KH):
        interleaved.append((whh_sb[:, k * 4 * U:(k + 1) * 4 * U],
                            whh_d[k * 128:(k + 1) * 128, :]))
    for k in range(KH):
        interleaved.append((hT_sb[:, k * B:(k + 1) * B],
                            hT_d[k * 128:(k + 1) * 128, :]))
    per_iter = (len(interleaved) + BS - 1) // BS

    stage_energy(0)

    # ---------- e_h row + emb transposes (PE order: after energy(0)) ----------
    eh_ps = ps.tile([1, BS], F32, tag="ps", name="ehps")
    for k in range(KH):
        nc.tensor.matmul(eh_ps[:], wh_sb[:, k:k + 1],
                         hTs_sb[:, k * BS:(k + 1) * BS],
                         start=(k == 0), stop=(k == KH - 1))
    ehrow_sb = cpool.tile([1, BS], F32, name="ehrow_sb")
    nc.vector.tensor_scalar_add(ehrow_sb[:], eh_ps[:], be_sb[0:1, 0:1])
    # rnn_in^T tile: k-chunks 0..15 = context^T (filled later), 16..19 = emb^T
    rnnT_sb = work.tile([128, KR * B], BF16, name="rnnT_sb")
    for ec in range(E // 128):
        pt = ps.tile([128, 128], F32, tag="ps", name="ptf")
        nc.tensor.transpose(pt[:], emb_sb[:, ec * 128:(ec + 1) * 128], idf[:])
        nc.vector.tensor_copy(
            out=rnnT_sb[:, (HC + ec) * B:(HC + ec + 1) * B], in_=pt[:])

    for b in range(BS):
        if b + 1 < BS:
            stage_energy(b + 1)
        expc = stage_soft(b)
        stage_ctx(b, expc)
        if b + 3 < BS:
            stage_load(b + 3)
        elif b + 3 == BS:
            pass
        for out_ap, in_ap in interleaved[b * per_iter:(b + 1) * per_iter]:
            nc.sync.dma_start(out=out_ap, in_=in_ap)

    # ---------- LSTM gates: emb + h_prev accumulation (collective-overlap) ----
    # All 4 gates share one PSUM bank [U, 4*B]; only the very first matmul
    # uses start=True, per-element has_written handles the rest.
    gps_all = ps.tile([U, 4 * B], F32, tag="ps", name="gps_all")
    first_mm = True
    for g in range(4):
        for k in range(HC, KR):
            nc.tensor.matmul(
                gps_all[:, g * B:(g + 1) * B],
                wih_sb[:, k * 4 * U + g * U: k * 4 * U + (g + 1) * U],
                rnnT_sb[:, k * B:(k + 1) * B], start=first_mm, stop=False,
                skip_group_check=True)
            first_mm = False
        for k in range(KH):
            nc.tensor.matmul(
                gps_all[:, g * B:(g + 1) * B],
                whh_sb[:, k * 4 * U + g * U: k * 4 * U + (g + 1) * U],
                hT_sb[:, k * B:(k + 1) * B], start=False, stop=False,
                skip_group_check=True)

    # ---------- gather context to all cores; build rnn_in^T ----------
    if SIM_SINGLE:
        for r in range(NCORES):
            nc.sync.dma_start(out=ctx_all[r * BS:(r + 1) * BS, :], in_=ctx_in[:])
    else:
        nc.gpsimd.collective_compute(
            "AllGather", mybir.AluOpType.bypass,
            replica_groups=[list(range(NCORES))],
            ins=[ctx_in.opt()], outs=[ctx_all.opt()])
    ctx_sb = work.tile([B, H2], BF16, name="ctx_sb")
    nc.sync.dma_start(out=ctx_sb[:], in_=ctx_all[:])
    for kc in range(HC):
        ptb = psc.tile([128, 128], BF16, tag="cps", name="ptb")
        nc.tensor.transpose(ptb[:], ctx_sb[:, kc * 128:(kc + 1) * 128], idb[:])
        nc.vector.tensor_copy(out=rnnT_sb[:, kc * B:(kc + 1) * B], in_=ptb[:])

    # ---------- FC weights load (after ctx-gather DMAs in queue order) ------
    nc.sync.dma_start(out=bfc_sb[:], in_=bfc_d[:])
    for k in range(KH):
        nc.sync.dma_start(out=wfc_sb[:, k * VS:(k + 1) * VS],
                          in_=wfc_d[k * 128:(k + 1) * 128, :])

    # ---------- LSTM gates: context accumulation + activations ----------
    gate_sb = []
    gate_fn = [AF.Sigmoid, AF.Sigmoid, AF.Tanh, AF.Sigmoid]
    for g in range(4):
        for k in range(HC):
            nc.tensor.matmul(
                gps_all[:, g * B:(g + 1) * B],
                wih_sb[:, k * 4 * U + g * U: k * 4 * U + (g + 1) * U],
                rnnT_sb[:, k * B:(k + 1) * B], start=False,
                stop=(g == 3 and k == HC - 1), skip_group_check=True)
    for g in range(4):
        gs = work.tile([U, B], F32, name=f"gate{g}", tag=f"gate{g}")
        nc.scalar.activation(gs[:], gps_all[:, g * B:(g + 1) * B], gate_fn[g],
                             bias=bias_sb[:, g:g + 1], scale=1.0)
        gate_sb.append(gs)

    fc_sb = work.tile([U, B], F32, name="fc_sb")
    nc.vector.tensor_mul(fc_sb[:], gate_sb[1][:], cT_sb[:])
    ig_sb = work.tile([U, B], F32, name="ig_sb")
    nc.vector.tensor_mul(ig_sb[:], gate_sb[0][:], gate_sb[2][:])
    cnew_sb = work.tile([U, B], F32, name="cnew_sb")
    nc.vector.tensor_add(cnew_sb[:], fc_sb[:], ig_sb[:])
    nc.sync.dma_start(out=cout_d[:], in_=cnew_sb[:])
    tanhc_sb = work.tile([U, B], F32, name="tanhc_sb")
    nc.scalar.activation(tanhc_sb[:], cnew_sb[:], AF.Tanh)
    hnew_sb = work.tile([U, B], F32, name="hnew_sb")
    nc.vector.tensor_mul(hnew_sb[:], gate_sb[3][:], tanhc_sb[:])
    nc.sync.dma_start(out=hout_d[:], in_=hnew_sb[:])

    # ---------- gather h_new to all cores ----------
    hnewb_sb = work.tile([U, B], BF16, name="hnewb_sb")
    nc.vector.tensor_copy(out=hnewb_sb[:], in_=hnew_sb[:])
    h_in = dpool.tile([U, B], BF16, name="h_in")
    h_all = dpool.tile([H, B], BF16, name="h_all")
    nc.sync.dma_start(out=h_in[:], in_=hnewb_sb[:])
    if SIM_SINGLE:
        for r in range(NCORES):
            nc.sync.dma_start(out=h_all[r * U:(r + 1) * U, :], in_=h_in[:])
    else:
        nc.gpsimd.collective_compute(
            "AllGather", mybir.AluOpType.bypass,
            replica_groups=[list(range(NCORES))],
            ins=[h_in.opt()], outs=[h_all.opt()])
    hTn_sb = work.tile([128, KH * B], BF16, name="hTn_sb")
    for k in range(KH):
        nc.sync.dma_start(out=hTn_sb[:, k * B:(k + 1) * B],
                          in_=h_all[k * 128:(k + 1) * 128, :])

    # ---------- FC (vocab-parallel): logits [b, v_slice] ----------
    for vc in range(NVC):
        v0 = vc * 512
        vn = min(512, VS - v0)
        f_ps = ps.tile([B, 512], F32, tag="ps", name="fps")
        nc.tensor.matmul(f_ps[:, :vn], ones_row[0:1, :],
                         bfc_sb[0:1, v0:v0 + vn], start=True, stop=False)
        for k in range(KH):
            nc.tensor.matmul(
                f_ps[:, :vn], hTn_sb[:, k * B:(k + 1) * B],
                wfc_sb[:, k * VS + v0: k * VS + v0 + vn],
                start=False, stop=(k == KH - 1))
        o_sb = rowp.tile([B, 512], F32, tag="osb", name="o_sb")
        nc.scalar.copy(o_sb[:, :vn], f_ps[:, :vn])
        nc.sync.dma_start(out=pred_d[:, v0:v0 + vn], in_=o_sb[:, :vn])


class _Runner:
    """Caches the jitted shard_map executable for a compiled Bass program."""

    def __init__(self, nc):
        bass2jax.install_neuronx_cc_hook()
        self.nc = nc
        assert nc.dbg_addr is None
        part_name = (nc.partition_id_tensor.name
                     if nc.partition_id_tensor else None)
        in_names, out_names, out_avals, zero_shapes = [], [], [], []
        for alloc in nc.m.functions[0].allocations:
            if not isinstance(alloc, mybir.MemoryLocationSet):
                continue
            name = alloc.memorylocations[0].name
            if alloc.kind == "ExternalInput":
                if name != part_name:
                    in_names.append(name)
            elif alloc.kind == "ExternalOutput":
                out_names.append(name)
                shape = tuple(alloc.tensor_shape)
                dtype = mybir.dt.np(alloc.dtype)
                out_avals.append(jax.core.ShapedArray(shape, dtype))
                zero_shapes.append((shape, dtype))
        self.in_names = in_names
        self.out_names = out_names
        self.zero_shapes = zero_shapes
        n_params = len(in_names)
        n_outs = len(out_names)
        all_names = list(in_names) + list(out_names)
        if part_name is not None:
            all_names.append(part_name)
        all_names = tuple(all_names)

        def _body(*args):
            operands = list(args)
            if part_name is not None:
                operands.append(bass2jax.partition_id_tensor())
            outs = bass2jax._bass_exec_p.bind(
                *operands, out_avals=tuple(out_avals), in_names=all_names,
                out_names=tuple(out_names), lowering_input_output_aliases=(),
                sim_require_finite=True, sim_require_nnan=True, nc=nc)
            return tuple(outs)

        devices = jax.devices()[:NCORES]
        assert len(devices) == NCORES
        self.mesh = Mesh(np.asarray(devices), ("core",))
        in_specs = (PartitionSpec("core"),) * (n_params + n_outs)
        out_specs = (PartitionSpec("core"),) * n_outs
        self._fn = jax.jit(
            shard_map(_body, mesh=self.mesh, in_specs=in_specs,
                      out_specs=out_specs, check_rep=False),
            donate_argnums=tuple(range(n_params, n_params + n_outs)),
            keep_unused=True)

    def concat_inputs(self, in_maps):
        return [np.concatenate([np.asarray(in_maps[c][n]) for c in range(NCORES)],
                               axis=0) for n in self.in_names]

    def device_put_inputs(self, in_maps):
        sh = NamedSharding(self.mesh, PartitionSpec("core"))
        return [jax.device_put(a, sh) for a in self.concat_inputs(in_maps)]

    def zeros(self):
        return [np.zeros((NCORES * s[0], *s[1:]), d) for s, d in self.zero_shapes]

    def run(self, concat_in):
        outs = self._fn(*concat_in, *self.zeros())
        return [o.block_until_ready() for o in outs]

    def results(self, outs):
        out_np = [np.asarray(o) for o in outs]
        return [
            {n: out_np[i].reshape(NCORES, *self.zero_shapes[i][0])[c]
             for i, n in enumerate(self.out_names)}
            for c in range(NCORES)
        ]


_RUNNERS = {}


def get_runner(reps=1):
    if reps not in _RUNNERS:
        _RUNNERS[reps] = _Runner(_build(reps))
    return _RUNNERS[reps]


def _prep_inputs(x, encoder_states, hidden, cell, emb_table, W_e, b_e,
                 W_ih, W_hh, b_ih, b_hh, W_fc, b_fc):
    """Host-side sharding + layout prep. Returns in_maps for the 8 cores."""
    x = np.asarray(x).astype(np.int32).reshape(B, 1)
    enc = np.asarray(encoder_states, dtype=np.float32)
    hidden = np.asarray(hidden, dtype=np.float32)
    cell = np.asarray(cell, dtype=np.float32)
    emb_table = np.ascontiguousarray(np.asarray(emb_table, dtype=np.float32))
    W_e = np.asarray(W_e, dtype=np.float32)
    b_e = np.asarray(b_e, dtype=np.float32)
    W_ih = np.asarray(W_ih, dtype=np.float32)
    W_hh = np.asarray(W_hh, dtype=np.float32)
    b_ih = np.asarray(b_ih, dtype=np.float32)
    b_hh = np.asarray(b_hh, dtype=np.float32)
    W_fc = np.asarray(W_fc, dtype=np.float32)
    b_fc = np.asarray(b_fc, dtype=np.float32)

    enc_q = enc.astype(ENC_NP)
    encN_all = np.ascontiguousarray(enc_q.transpose(1, 0, 2))  # [B, S, H2]
    encT_all = np.ascontiguousarray(enc_q.transpose(1, 2, 0))  # [B, H2, S]

    hT = np.ascontiguousarray(hidden[0].T)          # [H, B] f32
    hT_bf = hT.astype(BF)
    cT = np.ascontiguousarray(cell[0].T)            # [H, B] f32
    wh_t = np.ascontiguousarray(W_e[:H, 0].reshape(KH, 128).T).astype(BF)
    wenc_chunks = (W_e[H:, 0] * WENC_SCALE).reshape(HC, 128)
    if ENC_MODE == 'fp8':
        wenc_t = np.zeros((128, 16 * HC), dtype=F8NP)
        for k in range(HC // 2):
            wenc_t[:, 32 * k] = wenc_chunks[2 * k].astype(F8NP)
            wenc_t[:, 32 * k + 16] = wenc_chunks[2 * k + 1].astype(F8NP)
    else:
        wenc_t = np.ascontiguousarray(wenc_chunks.T).astype(BF)
    be = b_e.reshape(1, 1)
    bias_all = b_ih + b_hh
    idf = np.eye(128, dtype=np.float32)
    idb = np.eye(128, dtype=BF)
    one_row = np.ones((1, 128), dtype=BF)

    in_maps = []
    for c in range(NCORES):
        b0 = c * BS
        u0 = c * U
        v0 = c * VS
        rows = np.concatenate([np.arange(u0, u0 + U) + g * H for g in range(4)])
        in_maps.append({
            "encN": encN_all[b0:b0 + BS],
            "encT": encT_all[b0:b0 + BS],
            "hT": hT_bf,
            "hTs": np.ascontiguousarray(hT_bf[:, b0:b0 + BS]),
            "cTs": np.ascontiguousarray(cT[u0:u0 + U, :]),
            "xi": x,
            "embt": emb_table,
            "wenc": wenc_t,
            "wh": wh_t,
            "be": be,
            "wih": np.ascontiguousarray(W_ih[rows].T).astype(BF),
            "whh": np.ascontiguousarray(W_hh[rows].T).astype(BF),
            "bias": np.ascontiguousarray(bias_all[rows].reshape(4, U).T),
            "wfc": np.ascontiguousarray(W_fc[v0:v0 + VS].T).astype(BF),
            "bfc": b_fc[v0:v0 + VS].reshape(1, VS).astype(BF),
            "idf": idf,
            "idb": idb,
            "one": one_row,
        })
    return in_maps


def kernel(**inputs):
    runner = get_runner(reps=1)
    in_maps = _prep_inputs(**inputs)
    outs = runner.run(runner.concat_inputs(in_maps))
    return assemble(runner.results(outs))


def assemble(results):
    preds = np.concatenate([results[c]["pred"] for c in range(NCORES)], axis=1)
    hT_new = np.concatenate([results[c]["hout"] for c in range(NCORES)], axis=0)
    cT_new = np.concatenate([results[c]["cout"] for c in range(NCORES)], axis=0)
    h_new = np.ascontiguousarray(hT_new.T)[None]
    c_new = np.ascontiguousarray(cT_new.T)[None]
    return preds, h_new, c_new
